# revision 36
# baseline (speedup 1.0000x reference)
"""Trainium2 Bass kernel for nn_AttentionBlock (B=4, H=W=64, C=64, GROUPS=32).

Math (reference):
    hn = GroupNorm(x; gamma, beta, 32 groups, eps=1e-3)
    q = hn@wq+bq ; k = hn@wk+bk ; v = hn@wv+bv
    att = softmax(q k^T / 8) over the 4096 spatial positions
    out = x + (att @ v) @ wo + bo

Sharding: data-parallel, 2 cores per batch image, each core owns 2048 of the
4096 queries but holds the full key/value set for its batch. No collectives.

Per-core pipeline (fully fused on one NeuronCore):
  - xT [C=64, S=4096] arrives pre-transposed in bf16 (host does the cheap
    numpy transpose+cast), so channel-contraction matmuls need no on-chip
    transposes. x_q keeps the core's own query rows in fp32 for the residual.
  - GroupNorm stats via bn_stats/bn_aggr per channel on DVE, then tiny 0/1
    matmuls pair-combine channels into groups and expand back. The GN affine
    folds into the projection weights: W~ = diag(scale_c)@W, b~ = gnbias@W + b.
  - k-bias is dropped: it shifts each query's scores by a constant, which
    softmax cancels exactly.
  - Scores are computed transposed, ST[t, s] (keys on partitions), so exp(ST)
    feeds the att@v matmul directly as the moving operand - the attention
    matrix is never transposed. Score matmuls have K=64, so two key-chunks run
    CONCURRENTLY on the two halves of the PE array (row-tiling): chunk p rides
    rows 0:63 and chunk 16+p rides rows 64:127 (kT stores each group on its
    own partition half; qT carries every column on both halves).
  - Softmax is max-free: |score| <= ~3 for unit-normal inputs so exp cannot
    overflow, and softmax(x) == softmax(x - max) exactly.
  - exp() runs one ACT instruction per chunk-pair over a 2-bank PSUM tile to
    amortize the ~352-cycle activation pipeline latency.
  - v gets an appended ones-column so att@v also accumulates the softmax
    denominator l[s]. att@v is split into two K=64 halves accumulating into
    two PSUM banks (summed by one DVE add at stripe end): the halves run on
    opposite array halves, letting LDWEIGHTS overlap in-flight matmuls.
  - The output projection runs on the unnormalized accumulator ((O/l)@wo ==
    (O@wo)/l), with an extra wo column passing l through; one reciprocal +
    fused multiply-add applies softmax normalization, residual and bo.
"""

import numpy as np
import ml_dtypes

import concourse.tile as tile
from concourse import bacc, mybir
from concourse.bass_utils import run_bass_kernel_spmd

F32 = mybir.dt.float32
BF16 = mybir.dt.bfloat16
AF = mybir.ActivationFunctionType
ALU = mybir.AluOpType

B, H, W, C = 4, 64, 64, 64
S = H * W            # 4096 spatial positions per image
SQ = S // 2          # 2048 queries per core
EPS = 1e-3
N_CHUNK = S // 128   # 32 key chunks
NQ = SQ // 128       # 16 query chunks
N_STRIPE = SQ // 512  # 4 query stripes
SCALE = float(C) ** -0.5  # 0.125


def build_kernel():
    nc = bacc.Bacc("TRN2", target_bir_lowering=False, debug=False)

    xT_d = nc.dram_tensor("xT", [C, S], BF16, kind="ExternalInput")
    x_pm_d = nc.dram_tensor("x_pm", [128, N_CHUNK, 64], BF16, kind="ExternalInput")
    x_q = nc.dram_tensor("x_q", [SQ, C], F32, kind="ExternalInput")
    gamma = nc.dram_tensor("gamma", [C], F32, kind="ExternalInput")
    beta = nc.dram_tensor("beta", [C], F32, kind="ExternalInput")
    wq_d = nc.dram_tensor("wq", [C, C], F32, kind="ExternalInput")
    bq_d = nc.dram_tensor("bq", [C], F32, kind="ExternalInput")
    wk_d = nc.dram_tensor("wk", [C, C], F32, kind="ExternalInput")
    wv_d = nc.dram_tensor("wv", [C, C], F32, kind="ExternalInput")
    bv_d = nc.dram_tensor("bv", [C], F32, kind="ExternalInput")
    wo_d = nc.dram_tensor("wo", [C, C], F32, kind="ExternalInput")
    bo_d = nc.dram_tensor("bo", [C], F32, kind="ExternalInput")
    out_d = nc.dram_tensor("out", [SQ, C], F32, kind="ExternalOutput")

    with tile.TileContext(nc) as tc:
        _emit(nc, tc, xT_d.ap(), x_pm_d.ap(), x_q.ap(), gamma.ap(), beta.ap(),
              wq_d.ap(), bq_d.ap(), wk_d.ap(), wv_d.ap(), bv_d.ap(), wo_d.ap(),
              bo_d.ap(), out_d.ap())
    nc.compile()
    return nc


def _emit(nc, tc, xT_d, x_pm_d, x_q, gamma, beta, wq_d, bq_d, wk_d, wv_d,
          bv_d, wo_d, bo_d, out_d):
    from contextlib import ExitStack

    ctx = ExitStack()
    with ctx:
        const = ctx.enter_context(tc.tile_pool(name="const", bufs=1))
        big = ctx.enter_context(tc.tile_pool(name="big", bufs=1))
        tiny = ctx.enter_context(tc.tile_pool(name="tiny", bufs=1))

        # ---- big input DMAs first (sync/HWDGE ring), chunked so dependents
        # ---- can start early; partitions 64:127 mirror 0:63 for row-tiling ----
        xT = big.tile([128, S], BF16)
        x_pm = big.tile([128, N_CHUNK, 64], BF16)
        eng = [nc.sync, nc.scalar, nc.gpsimd, nc.sync]
        for i in range(4):
            eng[i].dma_start(out=xT[0:64, 1024 * i:1024 * (i + 1)],
                             in_=xT_d[:, 1024 * i:1024 * (i + 1)])
            eng[(i + 1) % 3].dma_start(out=x_pm[:, 8 * i:8 * (i + 1), :],
                                       in_=x_pm_d[:, 8 * i:8 * (i + 1), :])
        for i in range(4):
            nc.sync.dma_start(out=xT[64:128, 1024 * i:1024 * (i + 1)],
                              in_=xT[0:64, 1024 * i:1024 * (i + 1)])

        # ---- params via the scalar-engine HWDGE ring (parallel with sync) ----
        wq_aug = const.tile([65, 64], F32)   # [Wq ; bq]
        nc.scalar.dma_start(out=wq_aug[0:64, :], in_=wq_d)
        nc.scalar.dma_start(out=wq_aug[64:65, :], in_=bq_d.rearrange("(o c) -> o c", o=1))
        wk_sb = const.tile([128, 64], F32)
        nc.scalar.dma_start(out=wk_sb[0:64, :], in_=wk_d)
        nc.scalar.dma_start(out=wk_sb[64:128, :], in_=wk_d)
        wq_sb = const.tile([128, 64], F32)
        nc.scalar.dma_start(out=wq_sb[0:64, :], in_=wq_d)
        nc.scalar.dma_start(out=wq_sb[64:128, :], in_=wq_d)
        wv_sb = const.tile([128, 64], F32)
        nc.scalar.dma_start(out=wv_sb[0:64, :], in_=wv_d)
        nc.scalar.dma_start(out=wv_sb[64:128, :], in_=wv_d)
        wv_aug = const.tile([65, 65], F32)   # [Wv ; bv] plus e64 column
        nc.scalar.dma_start(out=wv_aug[0:64, 0:64], in_=wv_d)
        nc.scalar.dma_start(out=wv_aug[64:65, 0:64], in_=bv_d.rearrange("(o c) -> o c", o=1))
        nc.gpsimd.memset(wv_aug[0:64, 64:65], 0.0)
        nc.gpsimd.memset(wv_aug[64:65, 64:65], 1.0)
        # wo_aug = [wo ; bvo] plus e64 column that passes l through. Row 64
        # multiplies the l-row of the accumulator, so after the division by l
        # it contributes the constant row bvo = bv_total @ wo - this is how the
        # v-bias is applied without ever materializing it per-position.
        wo_aug = const.tile([65, 65], BF16)
        nc.gpsimd.dma_start(out=wo_aug[0:64, 0:64], in_=wo_d)  # SWDGE casts f32->bf16
        nc.gpsimd.memset(wo_aug[0:64, 64:65], 0.0)
        nc.gpsimd.memset(wo_aug[64:65, 64:65], 1.0)
        wo_sb = const.tile([64, 64], F32)
        nc.scalar.dma_start(out=wo_sb, in_=wo_d)
        gamma_col = const.tile([128, 1], F32)
        nc.scalar.dma_start(out=gamma_col[0:64, :], in_=gamma.rearrange("(c o) -> c o", o=1))
        nc.scalar.dma_start(out=gamma_col[64:128, :], in_=gamma.rearrange("(c o) -> c o", o=1))
        beta_col = const.tile([64, 1], F32)
        nc.scalar.dma_start(out=beta_col, in_=beta.rearrange("(c o) -> c o", o=1))
        bo_bcast = const.tile([128, 64], F32)
        nc.scalar.dma_start(out=bo_bcast, in_=bo_d.rearrange("(o c) -> o c", o=1).to_broadcast([128, 64]))

        zbias = const.tile([128, 1], F32)
        nc.gpsimd.memset(zbias, 0.0)
        ones_bf = const.tile([128, 1], BF16)
        nc.gpsimd.memset(ones_bf, 1.0)
        # exp is the only ACT table set this kernel uses (rsqrt is done with a
        # Newton iteration on DVE); preload it while waiting on input DMAs.
        scratch1 = const.tile([1, 1], F32)
        nc.scalar.activation(scratch1, zbias[0:1, :], AF.Exp, bias=0.0, scale=1.0)

        # pair matrices: p64h[c,g] = 1/8192 iff c//2 == g (sums -> group means);
        # p32x64[g,c] = 1 iff c//2 == g
        p64h = const.tile([64, 32], F32)
        nc.gpsimd.memset(p64h, 1.0 / 8192.0)
        nc.gpsimd.affine_select(out=p64h, in_=p64h, compare_op=ALU.is_ge,
                                fill=0.0, base=0, pattern=[[-2, 32]],
                                channel_multiplier=1)
        nc.gpsimd.affine_select(out=p64h, in_=p64h, compare_op=ALU.is_ge,
                                fill=0.0, base=1, pattern=[[2, 32]],
                                channel_multiplier=-1)
        p32x64 = const.tile([32, 64], F32)
        nc.gpsimd.memset(p32x64, 1.0)
        nc.gpsimd.affine_select(out=p32x64, in_=p32x64, compare_op=ALU.is_ge,
                                fill=0.0, base=0, pattern=[[1, 64]],
                                channel_multiplier=-2)
        nc.gpsimd.affine_select(out=p32x64, in_=p32x64, compare_op=ALU.is_ge,
                                fill=0.0, base=1, pattern=[[-1, 64]],
                                channel_multiplier=2)

        # ---- PSUM pools (8 banks: st 2x[128,1024] = 4, aux 4x one-bank) ----
        st_ps = ctx.enter_context(tc.tile_pool(name="st_ps", bufs=2, space="PSUM"))
        aux_ps = ctx.enter_context(tc.tile_pool(name="aux_ps", bufs=4, space="PSUM"))

        # ---- GroupNorm stats on PE: per-channel sum / sum-of-squares via
        # tiny ones-matmuls on the p-major bf16 copy (53ns issue rate, chases
        # the DMA chunks) - much lower latency than bn_stats on DVE and keeps
        # DVE free. p64h then turns sums into group means directly.
        xsq = big.tile([128, N_CHUNK, 64], BF16)
        for i in range(4):
            nc.vector.tensor_mul(xsq[:, 8 * i:8 * (i + 1), :],
                                 x_pm[:, 8 * i:8 * (i + 1), :],
                                 x_pm[:, 8 * i:8 * (i + 1), :])
        sums_ps = aux_ps.tile([64, 2], F32, tag="aux")
        for m in range(N_CHUNK):
            nc.tensor.matmul(sums_ps[:, 0:1], lhsT=x_pm[:, m, :], rhs=ones_bf,
                             start=(m == 0), stop=(m == N_CHUNK - 1))
        for m in range(N_CHUNK):
            nc.tensor.matmul(sums_ps[:, 1:2], lhsT=xsq[:, m, :], rhs=ones_bf,
                             start=(m == 0), stop=(m == N_CHUNK - 1))
        packed64 = tiny.tile([64, 2], F32)        # [sum_c, sum_sq_c]
        nc.vector.tensor_copy(packed64, sums_ps)
        gpair = aux_ps.tile([32, 2], F32, tag="aux")  # group [mean, E[x^2]]
        nc.tensor.matmul(gpair, lhsT=p64h, rhs=packed64)
        gm = tiny.tile([32, 2], F32)
        nc.vector.tensor_copy(gm, gpair)
        var = tiny.tile([32, 1], F32)
        nc.vector.tensor_mul(var, gm[:, 0:1], gm[:, 0:1])
        nc.vector.tensor_sub(var, gm[:, 1:2], var)
        nc.vector.tensor_scalar_add(var, var, EPS)
        # rstd = rsqrt(var) entirely on DVE: quake-style bit seed + 2 Newton
        # steps (rel err < 5e-6 for any positive input) - keeps the scalar
        # engine's activation tables untouched for exp. var acts as the
        # per-partition scalar operand so each step is 3 fused ops.
        U32 = mybir.dt.uint32
        magic = tiny.tile([32, 1], U32)
        nc.gpsimd.memset(magic, 0x5f3759df)
        packed32 = tiny.tile([32, 2], F32)        # [rstd_g | mean_g]
        nc.vector.tensor_copy(packed32[:, 1:2], gm[:, 0:1])
        ybits = tiny.tile([32, 1], U32)
        nc.vector.tensor_scalar(out=ybits, in0=var.bitcast(U32), scalar1=1,
                                scalar2=None, op0=ALU.logical_shift_right)
        nc.vector.tensor_sub(ybits, magic, ybits)
        y = ybits.bitcast(F32)
        c15 = tiny.tile([32, 1], F32)
        nc.gpsimd.memset(c15, 1.5)
        t1 = tiny.tile([32, 1], F32)
        for it in range(2):
            nc.vector.tensor_mul(t1, y, y)
            nc.vector.scalar_tensor_tensor(out=t1, in0=t1, scalar=var, in1=c15,
                                           op0=ALU.mult, op1=ALU.bypass)
            nc.vector.scalar_tensor_tensor(out=t1, in0=t1, scalar=-0.5, in1=c15,
                                           op0=ALU.mult, op1=ALU.add)
            dst = packed32[:, 0:1] if it == 1 else y
            nc.vector.tensor_mul(dst, y, t1)
        rstd = packed32[:, 0:1]
        chan = aux_ps.tile([128, 2], F32, tag="aux")  # expand groups->channels,
        nc.tensor.matmul(chan[0:64, :], lhsT=p32x64, rhs=packed32)  # both halves
        nc.tensor.matmul(chan[64:128, :], lhsT=p32x64, rhs=packed32,
                         tile_position=(0, 64))
        scale_col = tiny.tile([128, 1], F32)      # rstd_g * gamma_c (mirrored)
        nc.vector.tensor_mul(scale_col, chan[:, 0:1], gamma_col)
        gnbias = tiny.tile([65, 1], F32)          # beta - mean*scale, aug 1
        nc.vector.tensor_mul(gnbias[0:64, :], chan[0:64, 1:2], scale_col[0:64, :])
        nc.vector.tensor_sub(gnbias[0:64, :], beta_col, gnbias[0:64, :])
        nc.gpsimd.memset(gnbias[64:65, :], 1.0)

        # ---- fold GN into projection weights (both halves in one op) ----
        wq_sc = tiny.tile([128, 64], BF16)
        nc.vector.tensor_scalar_mul(wq_sc, wq_sb, scale_col)
        wk_sc = tiny.tile([128, 64], BF16)
        nc.vector.tensor_scalar_mul(wk_sc, wk_sb, scale_col)
        wv_sc = tiny.tile([128, 64], BF16)
        nc.vector.tensor_scalar_mul(wv_sc, wv_sb, scale_col)

        bqp = aux_ps.tile([128, 1], F32, tag="aux")  # total q bias, both halves
        nc.tensor.matmul(bqp[0:64, :], lhsT=wq_aug, rhs=gnbias)
        nc.tensor.matmul(bqp[64:128, :], lhsT=wq_aug, rhs=gnbias,
                         tile_position=(0, 64))
        bq_col = tiny.tile([128, 1], F32)
        nc.vector.tensor_copy(bq_col, bqp)
        # bvo row for wo_aug: bvo = (gnbias@Wv + bv) @ wo, bounced through HBM
        # to land on partition 64 (engines are lane-locked; DMA is not). This
        # only gates the first output projection, well off the critical path.
        bvcp = aux_ps.tile([65, 1], F32, tag="aux")
        nc.tensor.matmul(bvcp, lhsT=wv_aug, rhs=gnbias)
        bv_col = tiny.tile([64, 1], F32)
        nc.vector.tensor_copy(bv_col, bvcp[0:64, :])
        bvop = aux_ps.tile([1, 64], F32, tag="aux")
        nc.tensor.matmul(bvop, lhsT=bv_col, rhs=wo_sb)
        bvo_row = tiny.tile([1, 64], F32)
        nc.vector.tensor_copy(bvo_row, bvop)
        bvo_stage = nc.dram_tensor("bvo_stage", [64], F32).ap()
        nc.sync.dma_start(out=bvo_stage.rearrange("(o c) -> o c", o=1), in_=bvo_row)
        nc.gpsimd.dma_start(out=wo_aug[64:65, 0:64],
                            in_=bvo_stage.rearrange("(o c) -> o c", o=1))

        # ---- projections ----
        # Score matmuls pair key-chunk p (array rows 0:63) with chunk 16+p
        # (rows 64:127) - attention is order-invariant over keys - so kT keeps
        # chunks 0:15 on partitions 0:63 and chunks 16:31 on 64:127, and each
        # projection block lands directly on its half (output half chosen by
        # the tile_position column). qT needs every column on BOTH halves:
        # 4 tile-position variants cover the (stripe, half) grid pairwise-
        # concurrently. No SBUF mirror DMAs anywhere.
        kT = big.tile([128, SQ], BF16)
        qT = big.tile([128, SQ], BF16)

        # k/q projected in quad-groups: lo and hi array halves fill the two
        # partition halves of one [128,1024] tile (bank-staggered waves so
        # concurrent row-tiles never drain into the same bank), then ONE
        # full-width copy/bias-add moves both halves. kT's hi half holds
        # chunks 16:31 (see pairing note above); qT needs both halves of
        # every column, which this layout produces naturally.
        def kq_quad(dst, w_sc, lo_cols, hi_cols, bias):
            g = st_ps.tile([128, 1024], F32, tag="st")
            nc.tensor.matmul(g[0:64, 0:512], lhsT=w_sc[0:64, :],
                             rhs=xT[0:64, lo_cols:lo_cols + 512],
                             tile_position=(0, 0))
            nc.tensor.matmul(g[64:128, 512:1024], lhsT=w_sc[64:128, :],
                             rhs=xT[64:128, hi_cols + 512:hi_cols + 1024],
                             tile_position=(64, 64))
            nc.tensor.matmul(g[0:64, 512:1024], lhsT=w_sc[0:64, :],
                             rhs=xT[0:64, lo_cols + 512:lo_cols + 1024],
                             tile_position=(0, 0))
            nc.tensor.matmul(g[64:128, 0:512], lhsT=w_sc[64:128, :],
                             rhs=xT[64:128, hi_cols:hi_cols + 512],
                             tile_position=(64, 64))
            if bias is None:
                nc.scalar.copy(out=dst, in_=g)
            else:
                nc.vector.tensor_scalar_add(dst, g, bias)

        kq_quad(kT[:, 0:1024], wk_sc, 0, 2048, None)
        kq_quad(qT[:, 0:1024], wq_sc, 0, 0, bq_col)
        kq_quad(kT[:, 1024:2048], wk_sc, 1024, 3072, None)
        kq_quad(qT[:, 1024:2048], wq_sc, 1024, 1024, bq_col)

        # v in natural [t, c] layout; groups of 4 chunks {p, 8+p, 16+p, 24+p}
        # share one PSUM bank (quarter slices) and drain with one strided copy.
        # Chunks p,8+p ride array rows 0:63, 16+p,24+p rows 64:127 so the two
        # sub-pairs run concurrently. Column 64 = ones via one strided memset.
        v_big = big.tile([128, N_CHUNK, 65], BF16)
        nc.gpsimd.memset(v_big[:, :, 64:65], 1.0)
        # view: chunk (a*16 + b*8 + p) -> [p_, a, b, p, c]; lo-rows compute the
        # a=0 chunks into bank 0 of a 2-bank tile, hi-rows the a=1 chunks into
        # bank 1 (concurrent row-tiles must drain into distinct banks).
        v4 = v_big.rearrange("q (a b g) c -> q a b g c", a=2, b=2)

        def v_group(p):
            vga = aux_ps.tile([128, 2, 64], F32, tag="aux")
            vgb = aux_ps.tile([128, 2, 64], F32, tag="aux")
            for a, vg in ((0, vga), (1, vgb)):
                half = slice(64, 128) if a else slice(0, 64)
                tp = (64, 0) if a else (0, 0)
                for b in range(2):
                    ch = a * 16 + b * 8 + p
                    nc.tensor.matmul(vg[:, b, :],
                                     lhsT=xT[half, 128 * ch:128 * (ch + 1)],
                                     rhs=wv_sc[half, :], tile_position=tp)
            nc.vector.tensor_copy(v4[:, 0, :, p, 0:64], vga)
            nc.vector.tensor_copy(v4[:, 1, :, p, 0:64], vgb)

        for p in range(8):
            v_group(p)

        # ---- residual base: x + bo (needed only by epilogues) ----
        xq_sb = big.tile([128, NQ, 64], F32)
        nc.sync.dma_start(out=xq_sb, in_=x_q.rearrange("(m p) c -> p m c", p=128))
        xb_sb = big.tile([128, NQ, 64], F32)
        nc.vector.tensor_add(xb_sb, xq_sb,
                             bo_bcast.rearrange("p (o c) -> p o c", o=1).broadcast_to([128, NQ, 64]))

        # ---- main attention loop ----
        # Pairs of key chunks: the two K=64 score matmuls run concurrently on
        # the two row-halves of the PE array into the two banks of one PSUM
        # tile; exp covers both in one ACT instruction. att@v for each chunk
        # is split into two K=64 halves (lo/hi array rows) accumulating into
        # separate PSUM banks, summed once per stripe. All att@v work is
        # emitted one pair behind its exp so it never stalls the PE queue.
        p_pool = ctx.enter_context(tc.tile_pool(name="p_pool", bufs=6))
        ep_pool = ctx.enter_context(tc.tile_pool(name="ep_pool", bufs=3))
        N_PAIR = N_CHUNK // 2

        def emit_o(io, ot_lo, ot_hi, pt):
            # chunks arrive as 0, 16, 1, 17, ...: first is 0, last is 31
            first = io == 0
            last = io == N_CHUNK - 1
            nc.tensor.matmul(ot_lo, lhsT=v_big[0:64, io, :], rhs=pt[0:64, :],
                             tile_position=(0, 0), start=first, stop=last)
            nc.tensor.matmul(ot_hi, lhsT=v_big[64:128, io, :], rhs=pt[64:128, :],
                             tile_position=(64, 0), start=first, stop=last)

        def make_epilogue(j, ot_sb):
            last_stripe = j == N_STRIPE - 1

            def epi():
                res = ep_pool.tile([128, 4, 64], F32, tag="res", bufs=2)
                for m in range(4):
                    op = aux_ps.tile([128, 65], F32, tag="aux")
                    nc.tensor.matmul(op, lhsT=ot_sb[:, 128 * m:128 * (m + 1)],
                                     rhs=wo_aug)
                    rl = ep_pool.tile([128, 1], F32, tag="rl")
                    nc.vector.reciprocal(rl, op[:, 64:65])
                    nc.vector.scalar_tensor_tensor(out=res[:, m, :],
                                                   in0=op[:, 0:64],
                                                   scalar=rl,
                                                   in1=xb_sb[:, 4 * j + m, :],
                                                   op0=ALU.mult, op1=ALU.add)
                    if last_stripe:
                        # tail latency matters here: ship each chunk as soon
                        # as it exists so DMA completion overlaps the rest
                        base = 512 * j + 128 * m
                        nc.sync.dma_start(out=out_d[base:base + 128, :],
                                          in_=res[:, m, :])
                if not last_stripe:
                    nc.sync.dma_start(
                        out=out_d[512 * j:512 * (j + 1), :].rearrange("(m p) c -> p m c", p=128),
                        in_=res)
            return epi

        pending_epilogue = None
        for j in range(N_STRIPE):
            ot_lo = aux_ps.tile([65, 512], F32, tag="aux")
            ot_hi = aux_ps.tile([65, 512], F32, tag="aux")
            pts = {}
            for p in range(N_PAIR + 1):
                if p < N_PAIR:
                    st2 = st_ps.tile([128, 1024], F32, tag="st")
                    nc.tensor.matmul(st2[:, 0:512],
                                     lhsT=kT[0:64, 128 * p:128 * (p + 1)],
                                     rhs=qT[0:64, 512 * j:512 * (j + 1)],
                                     tile_position=(0, 0))
                    nc.tensor.matmul(st2[:, 512:1024],
                                     lhsT=kT[64:128, 128 * p:128 * (p + 1)],
                                     rhs=qT[64:128, 512 * j:512 * (j + 1)],
                                     tile_position=(64, 0))
                    pt = p_pool.tile([128, 1024], BF16, tag="p")
                    nc.scalar.activation(pt, st2, AF.Exp, bias=zbias, scale=SCALE)
                    pts[p] = pt
                if p == 3 and pending_epilogue is not None:
                    pending_epilogue()
                    pending_epilogue = None
                po = p - 1
                if po >= 0:
                    pt = pts.pop(po)
                    emit_o(po, ot_lo, ot_hi, pt[:, 0:512])
                    emit_o(16 + po, ot_lo, ot_hi, pt[:, 512:1024])
            # merge halves (+ l row); DVE may read only one PSUM input per op
            ot_sb = ep_pool.tile([65, 512], BF16, bufs=2, tag="ot_sb")
            nc.vector.tensor_copy(ot_sb, ot_lo)
            nc.vector.tensor_add(ot_sb, ot_sb, ot_hi)
            pending_epilogue = make_epilogue(j, ot_sb)
        pending_epilogue()


_NC_CACHE = {}


def _get_nc():
    if "nc" not in _NC_CACHE:
        _NC_CACHE["nc"] = build_kernel()
    return _NC_CACHE["nc"]


def build_in_maps(x, gamma, beta, wq, bq, wk, wv, bv, wo, bo):
    """Per-core NEFF input dicts plus (batch, rows) scatter info per core."""
    x = np.asarray(x, dtype=np.float32)
    shared = {
        "gamma": np.asarray(gamma, np.float32),
        "beta": np.asarray(beta, np.float32),
        "wq": np.asarray(wq, np.float32), "bq": np.asarray(bq, np.float32),
        "wk": np.asarray(wk, np.float32),
        "wv": np.asarray(wv, np.float32), "bv": np.asarray(bv, np.float32),
        "wo": np.asarray(wo, np.float32), "bo": np.asarray(bo, np.float32),
    }
    xf = x.reshape(B, S, C)
    in_maps = []
    scatter = []
    for core in range(8):
        b, h = core // 2, core % 2
        own = slice(h * SQ, (h + 1) * SQ)
        other = slice((1 - h) * SQ, (2 - h) * SQ)
        x_local = np.concatenate([xf[b][own], xf[b][other]], axis=0)
        x_bf = x_local.astype(ml_dtypes.bfloat16)
        in_maps.append({
            "xT": np.ascontiguousarray(x_bf.T),
            "x_pm": np.ascontiguousarray(x_bf.reshape(N_CHUNK, 128, C).transpose(1, 0, 2)),
            "x_q": np.ascontiguousarray(x_local[:SQ]),
            **shared,
        })
        scatter.append((b, np.arange(h * SQ, (h + 1) * SQ)))
    return in_maps, scatter


def _run(in_maps, scatter, **spmd_kwargs):
    nc = _get_nc()
    res = run_bass_kernel_spmd(nc, in_maps, core_ids=list(range(8)),
                               **spmd_kwargs)
    out = np.empty((B, S, C), np.float32)
    for core in range(8):
        b, rows = scatter[core]
        out[b][rows] = res.results[core]["out"]
    return out.reshape(B, H, W, C), res


def kernel(x, gamma, beta, wq, bq, wk, bk, wv, bv, wo, bo):
    # bk is provably a no-op: it shifts each query's scores by the constant
    # bk.q which softmax cancels, so it is not shipped to the device.
    in_maps, scatter = build_in_maps(x, gamma, beta, wq, bq, wk, wv, bv, wo, bo)
    out, _ = _run(in_maps, scatter)
    return out


# revision 37
# speedup vs baseline: 1.0370x; 1.0370x over previous
"""Trainium2 Bass kernel for nn_AttentionBlock (B=4, H=W=64, C=64, GROUPS=32).

Math (reference):
    hn = GroupNorm(x; gamma, beta, 32 groups, eps=1e-3)
    q = hn@wq+bq ; k = hn@wk+bk ; v = hn@wv+bv
    att = softmax(q k^T / 8) over the 4096 spatial positions
    out = x + (att @ v) @ wo + bo

Sharding: data-parallel, 2 cores per batch image, each core owns 2048 of the
4096 queries but holds the full key/value set for its batch. No collectives.

Per-core pipeline (fully fused on one NeuronCore):
  - xT [C=64, S=4096] arrives pre-transposed in bf16 (host does the cheap
    numpy transpose+cast), so channel-contraction matmuls need no on-chip
    transposes. x_q keeps the core's own query rows in fp32 for the residual.
  - GroupNorm stats via bn_stats/bn_aggr per channel on DVE, then tiny 0/1
    matmuls pair-combine channels into groups and expand back. The GN affine
    folds into the projection weights: W~ = diag(scale_c)@W, b~ = gnbias@W + b.
  - k-bias is dropped: it shifts each query's scores by a constant, which
    softmax cancels exactly.
  - Scores are computed transposed, ST[t, s] (keys on partitions), so exp(ST)
    feeds the att@v matmul directly as the moving operand - the attention
    matrix is never transposed. Score matmuls have K=64, so two key-chunks run
    CONCURRENTLY on the two halves of the PE array (row-tiling): chunk p rides
    rows 0:63 and chunk 16+p rides rows 64:127 (kT stores each group on its
    own partition half; qT carries every column on both halves).
  - Softmax is max-free: |score| <= ~3 for unit-normal inputs so exp cannot
    overflow, and softmax(x) == softmax(x - max) exactly.
  - exp() runs one ACT instruction per chunk-pair over a 2-bank PSUM tile to
    amortize the ~352-cycle activation pipeline latency.
  - v gets an appended ones-column so att@v also accumulates the softmax
    denominator l[s]. att@v is split into two K=64 halves accumulating into
    two PSUM banks (summed by one DVE add at stripe end): the halves run on
    opposite array halves, letting LDWEIGHTS overlap in-flight matmuls.
  - The output projection runs on the unnormalized accumulator ((O/l)@wo ==
    (O@wo)/l), with an extra wo column passing l through; one reciprocal +
    fused multiply-add applies softmax normalization, residual and bo.
"""

import numpy as np
import ml_dtypes

import concourse.tile as tile
from concourse import bacc, mybir
from concourse.bass_utils import run_bass_kernel_spmd

F32 = mybir.dt.float32
BF16 = mybir.dt.bfloat16
AF = mybir.ActivationFunctionType
ALU = mybir.AluOpType

B, H, W, C = 4, 64, 64, 64
S = H * W            # 4096 spatial positions per image
SQ = S // 2          # 2048 queries per core
EPS = 1e-3
N_CHUNK = S // 128   # 32 key chunks
NQ = SQ // 128       # 16 query chunks
N_STRIPE = SQ // 512  # 4 query stripes
SCALE = float(C) ** -0.5  # 0.125


def build_kernel():
    nc = bacc.Bacc("TRN2", target_bir_lowering=False, debug=False)

    xT_d = nc.dram_tensor("xT", [C, S], BF16, kind="ExternalInput")
    x_q = nc.dram_tensor("x_q", [SQ, C], F32, kind="ExternalInput")
    gamma = nc.dram_tensor("gamma", [C], F32, kind="ExternalInput")
    beta = nc.dram_tensor("beta", [C], F32, kind="ExternalInput")
    wq_d = nc.dram_tensor("wq", [C, C], F32, kind="ExternalInput")
    bq_d = nc.dram_tensor("bq", [C], F32, kind="ExternalInput")
    wk_d = nc.dram_tensor("wk", [C, C], F32, kind="ExternalInput")
    wv_d = nc.dram_tensor("wv", [C, C], F32, kind="ExternalInput")
    bv_d = nc.dram_tensor("bv", [C], F32, kind="ExternalInput")
    wo_d = nc.dram_tensor("wo", [C, C], F32, kind="ExternalInput")
    bo_d = nc.dram_tensor("bo", [C], F32, kind="ExternalInput")
    out_d = nc.dram_tensor("out", [SQ, C], F32, kind="ExternalOutput")

    with tile.TileContext(nc) as tc:
        _emit(nc, tc, xT_d.ap(), x_q.ap(), gamma.ap(), beta.ap(), wq_d.ap(),
              bq_d.ap(), wk_d.ap(), wv_d.ap(), bv_d.ap(), wo_d.ap(), bo_d.ap(),
              out_d.ap())
    nc.compile()
    return nc


def _emit(nc, tc, xT_d, x_q, gamma, beta, wq_d, bq_d, wk_d, wv_d, bv_d, wo_d,
          bo_d, out_d):
    from contextlib import ExitStack

    ctx = ExitStack()
    with ctx:
        const = ctx.enter_context(tc.tile_pool(name="const", bufs=1))
        big = ctx.enter_context(tc.tile_pool(name="big", bufs=1))
        tiny = ctx.enter_context(tc.tile_pool(name="tiny", bufs=1))

        # ---- big input DMAs first (sync/HWDGE ring), chunked so dependents
        # ---- can start early; partitions 64:127 mirror 0:63 for row-tiling ----
        xT = big.tile([128, S], BF16)
        eng = [nc.sync, nc.scalar, nc.gpsimd, nc.sync]
        for i in range(4):
            eng[i].dma_start(out=xT[0:64, 1024 * i:1024 * (i + 1)],
                             in_=xT_d[:, 1024 * i:1024 * (i + 1)])
        for i in range(4):
            nc.sync.dma_start(out=xT[64:128, 1024 * i:1024 * (i + 1)],
                              in_=xT[0:64, 1024 * i:1024 * (i + 1)])

        # ---- params via the scalar-engine HWDGE ring (parallel with sync) ----
        wq_aug = const.tile([65, 64], F32)   # [Wq ; bq]
        nc.scalar.dma_start(out=wq_aug[0:64, :], in_=wq_d)
        nc.scalar.dma_start(out=wq_aug[64:65, :], in_=bq_d.rearrange("(o c) -> o c", o=1))
        wk_sb = const.tile([128, 64], F32)
        nc.scalar.dma_start(out=wk_sb[0:64, :], in_=wk_d)
        nc.scalar.dma_start(out=wk_sb[64:128, :], in_=wk_d)
        wq_sb = const.tile([128, 64], F32)
        nc.scalar.dma_start(out=wq_sb[0:64, :], in_=wq_d)
        nc.scalar.dma_start(out=wq_sb[64:128, :], in_=wq_d)
        wv_sb = const.tile([128, 64], F32)
        nc.scalar.dma_start(out=wv_sb[0:64, :], in_=wv_d)
        nc.scalar.dma_start(out=wv_sb[64:128, :], in_=wv_d)
        wv_aug = const.tile([65, 65], F32)   # [Wv ; bv] plus e64 column
        nc.scalar.dma_start(out=wv_aug[0:64, 0:64], in_=wv_d)
        nc.scalar.dma_start(out=wv_aug[64:65, 0:64], in_=bv_d.rearrange("(o c) -> o c", o=1))
        nc.gpsimd.memset(wv_aug[0:64, 64:65], 0.0)
        nc.gpsimd.memset(wv_aug[64:65, 64:65], 1.0)
        # wo_aug = [wo ; bvo] plus e64 column that passes l through. Row 64
        # multiplies the l-row of the accumulator, so after the division by l
        # it contributes the constant row bvo = bv_total @ wo - this is how the
        # v-bias is applied without ever materializing it per-position.
        wo_aug = const.tile([65, 65], BF16)
        nc.gpsimd.dma_start(out=wo_aug[0:64, 0:64], in_=wo_d)  # SWDGE casts f32->bf16
        nc.gpsimd.memset(wo_aug[0:64, 64:65], 0.0)
        nc.gpsimd.memset(wo_aug[64:65, 64:65], 1.0)
        wo_sb = const.tile([64, 64], F32)
        nc.scalar.dma_start(out=wo_sb, in_=wo_d)
        gamma_col = const.tile([128, 1], F32)
        nc.scalar.dma_start(out=gamma_col[0:64, :], in_=gamma.rearrange("(c o) -> c o", o=1))
        nc.scalar.dma_start(out=gamma_col[64:128, :], in_=gamma.rearrange("(c o) -> c o", o=1))
        beta_col = const.tile([64, 1], F32)
        nc.scalar.dma_start(out=beta_col, in_=beta.rearrange("(c o) -> c o", o=1))
        bo_bcast = const.tile([128, 64], F32)
        nc.scalar.dma_start(out=bo_bcast, in_=bo_d.rearrange("(o c) -> o c", o=1).to_broadcast([128, 64]))

        zbias = const.tile([128, 1], F32)
        nc.gpsimd.memset(zbias, 0.0)
        # exp is the only ACT table set this kernel uses (rsqrt is done with a
        # Newton iteration on DVE); preload it while waiting on input DMAs.
        scratch1 = const.tile([1, 1], F32)
        nc.scalar.activation(scratch1, zbias[0:1, :], AF.Exp, bias=0.0, scale=1.0)

        # pair matrices: p64h[c,g] = 0.5 iff c//2 == g ; p32x64[g,c] = 1 iff c//2 == g
        p64h = const.tile([64, 32], F32)
        nc.gpsimd.memset(p64h, 0.5)
        nc.gpsimd.affine_select(out=p64h, in_=p64h, compare_op=ALU.is_ge,
                                fill=0.0, base=0, pattern=[[-2, 32]],
                                channel_multiplier=1)
        nc.gpsimd.affine_select(out=p64h, in_=p64h, compare_op=ALU.is_ge,
                                fill=0.0, base=1, pattern=[[2, 32]],
                                channel_multiplier=-1)
        p32x64 = const.tile([32, 64], F32)
        nc.gpsimd.memset(p32x64, 1.0)
        nc.gpsimd.affine_select(out=p32x64, in_=p32x64, compare_op=ALU.is_ge,
                                fill=0.0, base=0, pattern=[[1, 64]],
                                channel_multiplier=-2)
        nc.gpsimd.affine_select(out=p32x64, in_=p32x64, compare_op=ALU.is_ge,
                                fill=0.0, base=1, pattern=[[-1, 64]],
                                channel_multiplier=2)

        # ---- PSUM pools (8 banks: st 2x[128,1024] = 4, aux 4x one-bank) ----
        st_ps = ctx.enter_context(tc.tile_pool(name="st_ps", bufs=2, space="PSUM"))
        aux_ps = ctx.enter_context(tc.tile_pool(name="aux_ps", bufs=4, space="PSUM"))

        # ---- GroupNorm stats on DVE: per-channel mean/var over all 4096 ----
        bstats = tiny.tile([64, 8, 6], F32)
        for i in range(8):
            nc.vector.bn_stats(bstats[:, i, :], xT[0:64, 512 * i:512 * (i + 1)])
        mv = tiny.tile([64, 2], F32)
        nc.vector.bn_aggr(mv, bstats)
        packed64 = tiny.tile([64, 2], F32)        # [mean_c, E[x^2]_c]
        nc.vector.tensor_copy(packed64[:, 0:1], mv[:, 0:1])
        nc.vector.tensor_mul(packed64[:, 1:2], mv[:, 0:1], mv[:, 0:1])
        nc.vector.tensor_add(packed64[:, 1:2], packed64[:, 1:2], mv[:, 1:2])
        gpair = aux_ps.tile([32, 2], F32, tag="aux")  # group [mean, E[x^2]]
        nc.tensor.matmul(gpair, lhsT=p64h, rhs=packed64)
        gm = tiny.tile([32, 2], F32)
        nc.vector.tensor_copy(gm, gpair)
        var = tiny.tile([32, 1], F32)
        nc.vector.tensor_mul(var, gm[:, 0:1], gm[:, 0:1])
        nc.vector.tensor_sub(var, gm[:, 1:2], var)
        nc.vector.tensor_scalar_add(var, var, EPS)
        # rstd = rsqrt(var) entirely on DVE: quake-style bit seed + 2 Newton
        # steps (rel err < 5e-6 for any positive input) - keeps the scalar
        # engine's activation tables untouched for exp. var acts as the
        # per-partition scalar operand so each step is 3 fused ops.
        U32 = mybir.dt.uint32
        magic = tiny.tile([32, 1], U32)
        nc.gpsimd.memset(magic, 0x5f3759df)
        packed32 = tiny.tile([32, 2], F32)        # [rstd_g | mean_g]
        nc.vector.tensor_copy(packed32[:, 1:2], gm[:, 0:1])
        ybits = tiny.tile([32, 1], U32)
        nc.vector.tensor_scalar(out=ybits, in0=var.bitcast(U32), scalar1=1,
                                scalar2=None, op0=ALU.logical_shift_right)
        nc.vector.tensor_sub(ybits, magic, ybits)
        y = ybits.bitcast(F32)
        c15 = tiny.tile([32, 1], F32)
        nc.gpsimd.memset(c15, 1.5)
        t1 = tiny.tile([32, 1], F32)
        for it in range(2):
            nc.vector.tensor_mul(t1, y, y)
            nc.vector.scalar_tensor_tensor(out=t1, in0=t1, scalar=var, in1=c15,
                                           op0=ALU.mult, op1=ALU.bypass)
            nc.vector.scalar_tensor_tensor(out=t1, in0=t1, scalar=-0.5, in1=c15,
                                           op0=ALU.mult, op1=ALU.add)
            dst = packed32[:, 0:1] if it == 1 else y
            nc.vector.tensor_mul(dst, y, t1)
        rstd = packed32[:, 0:1]
        chan = aux_ps.tile([128, 2], F32, tag="aux")  # expand groups->channels,
        nc.tensor.matmul(chan[0:64, :], lhsT=p32x64, rhs=packed32)  # both halves
        nc.tensor.matmul(chan[64:128, :], lhsT=p32x64, rhs=packed32,
                         tile_position=(0, 64))
        scale_col = tiny.tile([128, 1], F32)      # rstd_g * gamma_c (mirrored)
        nc.vector.tensor_mul(scale_col, chan[:, 0:1], gamma_col)
        gnbias = tiny.tile([65, 1], F32)          # beta - mean*scale, aug 1
        nc.vector.tensor_mul(gnbias[0:64, :], chan[0:64, 1:2], scale_col[0:64, :])
        nc.vector.tensor_sub(gnbias[0:64, :], beta_col, gnbias[0:64, :])
        nc.gpsimd.memset(gnbias[64:65, :], 1.0)

        # ---- fold GN into projection weights (both halves in one op) ----
        wq_sc = tiny.tile([128, 64], BF16)
        nc.vector.tensor_scalar_mul(wq_sc, wq_sb, scale_col)
        wk_sc = tiny.tile([128, 64], BF16)
        nc.vector.tensor_scalar_mul(wk_sc, wk_sb, scale_col)
        wv_sc = tiny.tile([128, 64], BF16)
        nc.vector.tensor_scalar_mul(wv_sc, wv_sb, scale_col)

        bqp = aux_ps.tile([128, 1], F32, tag="aux")  # total q bias, both halves
        nc.tensor.matmul(bqp[0:64, :], lhsT=wq_aug, rhs=gnbias)
        nc.tensor.matmul(bqp[64:128, :], lhsT=wq_aug, rhs=gnbias,
                         tile_position=(0, 64))
        bq_col = tiny.tile([128, 1], F32)
        nc.vector.tensor_copy(bq_col, bqp)
        # bvo row for wo_aug: bvo = (gnbias@Wv + bv) @ wo, bounced through HBM
        # to land on partition 64 (engines are lane-locked; DMA is not). This
        # only gates the first output projection, well off the critical path.
        bvcp = aux_ps.tile([65, 1], F32, tag="aux")
        nc.tensor.matmul(bvcp, lhsT=wv_aug, rhs=gnbias)
        bv_col = tiny.tile([64, 1], F32)
        nc.vector.tensor_copy(bv_col, bvcp[0:64, :])
        bvop = aux_ps.tile([1, 64], F32, tag="aux")
        nc.tensor.matmul(bvop, lhsT=bv_col, rhs=wo_sb)
        bvo_row = tiny.tile([1, 64], F32)
        nc.vector.tensor_copy(bvo_row, bvop)
        bvo_stage = nc.dram_tensor("bvo_stage", [64], F32).ap()
        nc.sync.dma_start(out=bvo_stage.rearrange("(o c) -> o c", o=1), in_=bvo_row)
        nc.gpsimd.dma_start(out=wo_aug[64:65, 0:64],
                            in_=bvo_stage.rearrange("(o c) -> o c", o=1))

        # ---- projections ----
        # Score matmuls pair key-chunk p (array rows 0:63) with chunk 16+p
        # (rows 64:127) - attention is order-invariant over keys - so kT keeps
        # chunks 0:15 on partitions 0:63 and chunks 16:31 on 64:127, and each
        # projection block lands directly on its half (output half chosen by
        # the tile_position column). qT needs every column on BOTH halves:
        # 4 tile-position variants cover the (stripe, half) grid pairwise-
        # concurrently. No SBUF mirror DMAs anywhere.
        kT = big.tile([128, SQ], BF16)
        qT = big.tile([128, SQ], BF16)

        # k/q projected in quad-groups: lo and hi array halves fill the two
        # partition halves of one [128,1024] tile (bank-staggered waves so
        # concurrent row-tiles never drain into the same bank), then ONE
        # full-width copy/bias-add moves both halves. kT's hi half holds
        # chunks 16:31 (see pairing note above); qT needs both halves of
        # every column, which this layout produces naturally.
        def kq_quad(dst, w_sc, lo_cols, hi_cols, bias):
            g = st_ps.tile([128, 1024], F32, tag="st")
            nc.tensor.matmul(g[0:64, 0:512], lhsT=w_sc[0:64, :],
                             rhs=xT[0:64, lo_cols:lo_cols + 512],
                             tile_position=(0, 0))
            nc.tensor.matmul(g[64:128, 512:1024], lhsT=w_sc[64:128, :],
                             rhs=xT[64:128, hi_cols + 512:hi_cols + 1024],
                             tile_position=(64, 64))
            nc.tensor.matmul(g[0:64, 512:1024], lhsT=w_sc[0:64, :],
                             rhs=xT[0:64, lo_cols + 512:lo_cols + 1024],
                             tile_position=(0, 0))
            nc.tensor.matmul(g[64:128, 0:512], lhsT=w_sc[64:128, :],
                             rhs=xT[64:128, hi_cols:hi_cols + 512],
                             tile_position=(64, 64))
            if bias is None:
                nc.scalar.copy(out=dst, in_=g)
            else:
                nc.vector.tensor_scalar_add(dst, g, bias)

        kq_quad(kT[:, 0:1024], wk_sc, 0, 2048, None)
        kq_quad(qT[:, 0:1024], wq_sc, 0, 0, bq_col)
        kq_quad(kT[:, 1024:2048], wk_sc, 1024, 3072, None)
        kq_quad(qT[:, 1024:2048], wq_sc, 1024, 1024, bq_col)

        # v in natural [t, c] layout; groups of 4 chunks {p, 8+p, 16+p, 24+p}
        # share one PSUM bank (quarter slices) and drain with one strided copy.
        # Chunks p,8+p ride array rows 0:63, 16+p,24+p rows 64:127 so the two
        # sub-pairs run concurrently. Column 64 = ones via one strided memset.
        v_big = big.tile([128, N_CHUNK, 65], BF16)
        nc.gpsimd.memset(v_big[:, :, 64:65], 1.0)
        # view: chunk (a*16 + b*8 + p) -> [p_, a, b, p, c]; lo-rows compute the
        # a=0 chunks into bank 0 of a 2-bank tile, hi-rows the a=1 chunks into
        # bank 1 (concurrent row-tiles must drain into distinct banks).
        v4 = v_big.rearrange("q (a b g) c -> q a b g c", a=2, b=2)

        def v_group(p):
            vga = aux_ps.tile([128, 2, 64], F32, tag="aux")
            vgb = aux_ps.tile([128, 2, 64], F32, tag="aux")
            for a, vg in ((0, vga), (1, vgb)):
                half = slice(64, 128) if a else slice(0, 64)
                tp = (64, 0) if a else (0, 0)
                for b in range(2):
                    ch = a * 16 + b * 8 + p
                    nc.tensor.matmul(vg[:, b, :],
                                     lhsT=xT[half, 128 * ch:128 * (ch + 1)],
                                     rhs=wv_sc[half, :], tile_position=tp)
            nc.vector.tensor_copy(v4[:, 0, :, p, 0:64], vga)
            nc.vector.tensor_copy(v4[:, 1, :, p, 0:64], vgb)

        for p in range(8):
            v_group(p)

        # ---- residual base: x + bo (needed only by epilogues) ----
        xq_sb = big.tile([128, NQ, 64], F32)
        nc.sync.dma_start(out=xq_sb, in_=x_q.rearrange("(m p) c -> p m c", p=128))
        xb_sb = big.tile([128, NQ, 64], F32)
        nc.vector.tensor_add(xb_sb, xq_sb,
                             bo_bcast.rearrange("p (o c) -> p o c", o=1).broadcast_to([128, NQ, 64]))

        # ---- main attention loop ----
        # Pairs of key chunks: the two K=64 score matmuls run concurrently on
        # the two row-halves of the PE array into the two banks of one PSUM
        # tile; exp covers both in one ACT instruction. att@v for each chunk
        # is split into two K=64 halves (lo/hi array rows) accumulating into
        # separate PSUM banks, summed once per stripe. All att@v work is
        # emitted one pair behind its exp so it never stalls the PE queue.
        p_pool = ctx.enter_context(tc.tile_pool(name="p_pool", bufs=6))
        ep_pool = ctx.enter_context(tc.tile_pool(name="ep_pool", bufs=3))
        N_PAIR = N_CHUNK // 2

        def emit_o(io, ot_lo, ot_hi, pt):
            # chunks arrive as 0, 16, 1, 17, ...: first is 0, last is 31
            first = io == 0
            last = io == N_CHUNK - 1
            nc.tensor.matmul(ot_lo, lhsT=v_big[0:64, io, :], rhs=pt[0:64, :],
                             tile_position=(0, 0), start=first, stop=last)
            nc.tensor.matmul(ot_hi, lhsT=v_big[64:128, io, :], rhs=pt[64:128, :],
                             tile_position=(64, 0), start=first, stop=last)

        def make_epilogue(j, ot_sb):
            last_stripe = j == N_STRIPE - 1

            def epi():
                res = ep_pool.tile([128, 4, 64], F32, tag="res", bufs=2)
                for m in range(4):
                    op = aux_ps.tile([128, 65], F32, tag="aux")
                    nc.tensor.matmul(op, lhsT=ot_sb[:, 128 * m:128 * (m + 1)],
                                     rhs=wo_aug)
                    rl = ep_pool.tile([128, 1], F32, tag="rl")
                    nc.vector.reciprocal(rl, op[:, 64:65])
                    nc.vector.scalar_tensor_tensor(out=res[:, m, :],
                                                   in0=op[:, 0:64],
                                                   scalar=rl,
                                                   in1=xb_sb[:, 4 * j + m, :],
                                                   op0=ALU.mult, op1=ALU.add)
                    if last_stripe:
                        # tail latency matters here: ship each chunk as soon
                        # as it exists so DMA completion overlaps the rest
                        base = 512 * j + 128 * m
                        nc.sync.dma_start(out=out_d[base:base + 128, :],
                                          in_=res[:, m, :])
                if not last_stripe:
                    nc.sync.dma_start(
                        out=out_d[512 * j:512 * (j + 1), :].rearrange("(m p) c -> p m c", p=128),
                        in_=res)
            return epi

        pending_epilogue = None
        for j in range(N_STRIPE):
            ot_lo = aux_ps.tile([65, 512], F32, tag="aux")
            ot_hi = aux_ps.tile([65, 512], F32, tag="aux")
            pts = {}
            for p in range(N_PAIR + 1):
                if p < N_PAIR:
                    st2 = st_ps.tile([128, 1024], F32, tag="st")
                    nc.tensor.matmul(st2[:, 0:512],
                                     lhsT=kT[0:64, 128 * p:128 * (p + 1)],
                                     rhs=qT[0:64, 512 * j:512 * (j + 1)],
                                     tile_position=(0, 0))
                    nc.tensor.matmul(st2[:, 512:1024],
                                     lhsT=kT[64:128, 128 * p:128 * (p + 1)],
                                     rhs=qT[64:128, 512 * j:512 * (j + 1)],
                                     tile_position=(64, 0))
                    pt = p_pool.tile([128, 1024], BF16, tag="p")
                    nc.scalar.activation(pt, st2, AF.Exp, bias=zbias, scale=SCALE)
                    pts[p] = pt
                if p == 3 and pending_epilogue is not None:
                    pending_epilogue()
                    pending_epilogue = None
                po = p - 1
                if po >= 0:
                    pt = pts.pop(po)
                    emit_o(po, ot_lo, ot_hi, pt[:, 0:512])
                    emit_o(16 + po, ot_lo, ot_hi, pt[:, 512:1024])
            # merge halves (+ l row); DVE may read only one PSUM input per op
            ot_sb = ep_pool.tile([65, 512], BF16, bufs=2, tag="ot_sb")
            nc.vector.tensor_copy(ot_sb, ot_lo)
            nc.vector.tensor_add(ot_sb, ot_sb, ot_hi)
            pending_epilogue = make_epilogue(j, ot_sb)
        pending_epilogue()


_NC_CACHE = {}


def _get_nc():
    if "nc" not in _NC_CACHE:
        _NC_CACHE["nc"] = build_kernel()
    return _NC_CACHE["nc"]


def build_in_maps(x, gamma, beta, wq, bq, wk, wv, bv, wo, bo):
    """Per-core NEFF input dicts plus (batch, rows) scatter info per core."""
    x = np.asarray(x, dtype=np.float32)
    shared = {
        "gamma": np.asarray(gamma, np.float32),
        "beta": np.asarray(beta, np.float32),
        "wq": np.asarray(wq, np.float32), "bq": np.asarray(bq, np.float32),
        "wk": np.asarray(wk, np.float32),
        "wv": np.asarray(wv, np.float32), "bv": np.asarray(bv, np.float32),
        "wo": np.asarray(wo, np.float32), "bo": np.asarray(bo, np.float32),
    }
    xf = x.reshape(B, S, C)
    in_maps = []
    scatter = []
    for core in range(8):
        b, h = core // 2, core % 2
        own = slice(h * SQ, (h + 1) * SQ)
        other = slice((1 - h) * SQ, (2 - h) * SQ)
        x_local = np.concatenate([xf[b][own], xf[b][other]], axis=0)
        in_maps.append({
            "xT": np.ascontiguousarray(x_local.T).astype(ml_dtypes.bfloat16),
            "x_q": np.ascontiguousarray(x_local[:SQ]),
            **shared,
        })
        scatter.append((b, np.arange(h * SQ, (h + 1) * SQ)))
    return in_maps, scatter


def _run(in_maps, scatter, **spmd_kwargs):
    nc = _get_nc()
    res = run_bass_kernel_spmd(nc, in_maps, core_ids=list(range(8)),
                               **spmd_kwargs)
    out = np.empty((B, S, C), np.float32)
    for core in range(8):
        b, rows = scatter[core]
        out[b][rows] = res.results[core]["out"]
    return out.reshape(B, H, W, C), res


def kernel(x, gamma, beta, wq, bq, wk, bk, wv, bv, wo, bo):
    # bk is provably a no-op: it shifts each query's scores by the constant
    # bk.q which softmax cancels, so it is not shipped to the device.
    in_maps, scatter = build_in_maps(x, gamma, beta, wq, bq, wk, wv, bv, wo, bo)
    out, _ = _run(in_maps, scatter)
    return out
